# revision 25
# baseline (speedup 1.0000x reference)
"""EntmaxBisectLoss (alpha=1.5, 24 bisection iters, reduction=sum) on 8 TRN2 cores.

Data-parallel over rows (512/core). Per 128-row chunk:
  1. Stream X once: row max (DVE).
  2. Newton iterations for the entmax root tau*: each iteration streams X and
     accumulates S1 = sum relu(x - tau), S2 = sum relu(x - tau)^2 via DVE
     tensor_scalar accumulate + ACT Square accumulate. Newton from tau_lo
     converges monotonically (f convex decreasing).
  3. The reference's 24 fp32 bisection steps are emulated arithmetically
     against tau* (exact fp32 halving; p evaluated at the last midpoint).
  4. Final stream: Fenchel-Young loss sums (sum p, sum p^1.5, sum p*x,
     X[target] via one-hot accumulation). Per-row losses DMA'd out; host sums.
"""

import numpy as np

P = 128
V = 32000
N = 4096
NCORES = 8
RPC = N // NCORES
NCH = RPC // P
SUB = 8
SUBW = V // SUB          # 4000
NEWT = 11
C1 = np.float32((1.0 / V) ** 0.5)

_CACHE = {}


def _build():
    import concourse.bass as bass
    import concourse.bacc as bacc
    import concourse.mybir as mybir
    from concourse.tile import TileContext

    f32 = mybir.dt.float32
    i32 = mybir.dt.int32
    X_ = mybir.AxisListType.X
    Op = mybir.AluOpType
    Act = mybir.ActivationFunctionType

    nc = bacc.Bacc()
    Xd = nc.declare_dram_parameter("X", [RPC, V], f32, isOutput=False)
    Td = nc.declare_dram_parameter("target", [RPC], i32, isOutput=False)
    Ld = nc.declare_dram_parameter("loss_rows", [RPC], f32, isOutput=True)
    Dbg = nc.declare_dram_parameter("dbg", [RPC, 8], f32, isOutput=True)

    with TileContext(nc) as tc:
        with (
            tc.tile_pool(name="const", bufs=1) as cpool,
            tc.tile_pool(name="stream", bufs=3) as spool,
            tc.tile_pool(name="work", bufs=3) as wpool,
            tc.tile_pool(name="keep", bufs=1) as kpool,
            tc.tile_pool(name="small", bufs=2) as mpool,
        ):
            vio_np = np.broadcast_to(
                np.arange(SUBW, dtype=np.float32), (P, SUBW))
            vio_dram = nc.inline_tensor(np.ascontiguousarray(vio_np), name="viota")
            viota = cpool.tile([P, SUBW], f32, tag="viota")
            nc.sync.dma_start(out=viota[:], in_=vio_dram[:])

            tgt_sb = kpool.tile([P, NCH], i32, tag="tgt")
            nc.sync.dma_start(out=tgt_sb[:], in_=Td[:].rearrange("(c p) -> p c", p=P))
            tgt_f = kpool.tile([P, NCH], f32, tag="tgtf")
            nc.vector.tensor_copy(tgt_f[:], tgt_sb[:])

            rmaxS = kpool.tile([P, NCH], f32, tag="rmaxS")
    
            tauMS = kpool.tile([P, NCH], f32, tag="tauMS")
            tauS = kpool.tile([P, NCH], f32, tag="tauS")
            XtS = kpool.tile([P, NCH], f32, tag="XtS")
            SpS = kpool.tile([P, NCH], f32, tag="SpS")
            Sp3S = kpool.tile([P, NCH], f32, tag="Sp3S")
            SpXS = kpool.tile([P, NCH], f32, tag="SpXS")

            def stream_chunk(c, body):
                for s in range(SUB):
                    xt = spool.tile([P, SUBW], f32, tag="xt")
                    nc.sync.dma_start(
                        out=xt[:],
                        in_=Xd[c * P:(c + 1) * P, s * SUBW:(s + 1) * SUBW])
                    body(s, xt)

            for c in range(NCH):
                # ---- pass 1: row max
                pmax = mpool.tile([P, SUB], f32, tag="pmax")

                def bmax(s, xt, pmax=pmax):
                    nc.vector.tensor_reduce(
                        out=pmax[:, s:s + 1], in_=xt[:], axis=X_, op=Op.max)
                stream_chunk(c, bmax)
                rmax = mpool.tile([P, 1], f32, tag="rmax")
                nc.vector.tensor_reduce(out=rmax[:], in_=pmax[:], axis=X_, op=Op.max)
                nc.vector.tensor_copy(rmaxS[:, c:c + 1], rmax[:])

                # ---- Newton iterations from tau_lo0 = rmax - 2 (X units)
                tau = mpool.tile([P, 1], f32, tag="tau")
                nc.vector.tensor_scalar(tau[:], rmax[:], 2.0, None, op0=Op.subtract)
                for it in range(NEWT):
                    s1p = mpool.tile([P, SUB], f32, tag="s1p")
                    s2p = mpool.tile([P, SUB], f32, tag="s2p")

                    def bnewt(s, xt, s1p=s1p, s2p=s2p, tau=tau):
                        r = wpool.tile([P, SUBW], f32, tag="r")
                        nc.vector.tensor_scalar(
                            r[:], xt[:], tau[:, 0:1], 0.0, op0=Op.subtract,
                            op1=Op.max)
                        nc.vector.tensor_reduce(
                            out=s1p[:, s:s + 1], in_=r[:], axis=X_, op=Op.add)
                        sq = wpool.tile([P, SUBW], f32, tag="sq")
                        nc.scalar.activation(
                            sq[:], r[:], Act.Square, accum_out=s2p[:, s:s + 1])
                    stream_chunk(c, bnewt)
                    S1 = mpool.tile([P, 1], f32, tag="S1")
                    nc.vector.tensor_reduce(out=S1[:], in_=s1p[:], axis=X_, op=Op.add)
                    S2 = mpool.tile([P, 1], f32, tag="S2")
                    nc.vector.tensor_reduce(out=S2[:], in_=s2p[:], axis=X_, op=Op.add)
                    num = mpool.tile([P, 1], f32, tag="num")
                    nc.vector.tensor_scalar(num[:], S2[:], 4.0, None, op0=Op.subtract)
                    den = mpool.tile([P, 1], f32, tag="den")
                    nc.vector.tensor_scalar(den[:], S1[:], 2.0, None, op0=Op.mult)
                    rec = mpool.tile([P, 1], f32, tag="rec")
                    nc.vector.reciprocal(rec[:], den[:])
                    stp = mpool.tile([P, 1], f32, tag="stp")
                    nc.vector.tensor_tensor(out=stp[:], in0=num[:], in1=rec[:], op=Op.mult)
                    nc.vector.tensor_tensor(out=tau[:], in0=tau[:], in1=stp[:], op=Op.add)

                # ---- emulated reference bisection (Xs units) -> tau_m
                rms = mpool.tile([P, 1], f32, tag="rms")
                nc.vector.tensor_scalar(rms[:], rmax[:], 0.5, None, op0=Op.mult)
                lo = mpool.tile([P, 1], f32, tag="lo")
                nc.vector.tensor_scalar(lo[:], rms[:], 1.0, None, op0=Op.subtract)
                hi = mpool.tile([P, 1], f32, tag="hi")
                nc.vector.tensor_scalar(hi[:], rms[:], float(C1), None, op0=Op.subtract)
                dm = mpool.tile([P, 1], f32, tag="dm")
                nc.vector.tensor_tensor(out=dm[:], in0=hi[:], in1=lo[:], op=Op.subtract)
                that = mpool.tile([P, 1], f32, tag="that")
                nc.vector.tensor_scalar(that[:], tau[:], 0.5, None, op0=Op.mult)
                tm = mpool.tile([P, 1], f32, tag="tm")
                for i in range(24):
                    nc.vector.tensor_scalar(dm[:], dm[:], 0.5, None, op0=Op.mult)
                    nc.vector.tensor_tensor(out=tm[:], in0=lo[:], in1=dm[:], op=Op.add)
                    if i < 23:
                        acc = mpool.tile([P, 1], mybir.dt.uint8, tag="acc")
                        nc.vector.tensor_tensor(
                            out=acc[:], in0=tm[:], in1=that[:], op=Op.is_le)
                        nc.vector.copy_predicated(lo[:], acc[:], tm[:])
                tauM = mpool.tile([P, 1], f32, tag="tauM")
                nc.vector.tensor_scalar(tauM[:], tm[:], 2.0, None, op0=Op.mult)
                nc.vector.tensor_copy(tauMS[:, c:c + 1], tauM[:])
                nc.vector.tensor_copy(tauS[:, c:c + 1], tau[:])

                # ---- final stream: loss sums at tau_m
                spp = mpool.tile([P, SUB], f32, tag="spp")
                p3p = mpool.tile([P, SUB], f32, tag="p3p")
                sxp = mpool.tile([P, SUB], f32, tag="sxp")
                xtp = mpool.tile([P, SUB], f32, tag="xtp")
                tcol = mpool.tile([P, 1], f32, tag="tcol")
                nc.vector.tensor_copy(tcol[:], tgt_f[:, c:c + 1])

                def bfin(s, xt, spp=spp, p3p=p3p, sxp=sxp, xtp=xtp,
                         tauM=tauM, tcol=tcol):
                    r = wpool.tile([P, SUBW], f32, tag="r")
                    nc.vector.tensor_scalar(
                        r[:], xt[:], tauM[:, 0:1], 0.0, op0=Op.subtract, op1=Op.max)
                    sq = wpool.tile([P, SUBW], f32, tag="sq")
                    nc.scalar.activation(
                        sq[:], r[:], Act.Square, accum_out=spp[:, s:s + 1])
                    junk = wpool.tile([P, SUBW], f32, tag="junk")
                    nc.vector.tensor_tensor(out=junk[:], in0=sq[:], in1=r[:], op=Op.mult)
                    nc.vector.tensor_reduce(
                        out=p3p[:, s:s + 1], in_=junk[:], axis=X_, op=Op.add)
                    junk2 = wpool.tile([P, SUBW], f32, tag="junk")
                    nc.vector.tensor_tensor(out=junk2[:], in0=sq[:], in1=xt[:], op=Op.mult)
                    nc.vector.tensor_reduce(
                        out=sxp[:, s:s + 1], in_=junk2[:], axis=X_, op=Op.add)
                    # one-hot target: (viota + s*SUBW) == tgt
                    eq2 = wpool.tile([P, SUBW], f32, tag="junk")
                    nc.vector.tensor_scalar(
                        eq2[:], viota[:], float(s * SUBW), tcol[:, 0:1],
                        op0=Op.add, op1=Op.is_equal)
                    junk3 = wpool.tile([P, SUBW], f32, tag="junk")
                    nc.vector.tensor_tensor(out=junk3[:], in0=eq2[:], in1=xt[:], op=Op.mult)
                    nc.vector.tensor_reduce(
                        out=xtp[:, s:s + 1], in_=junk3[:], axis=X_, op=Op.add)
                stream_chunk(c, bfin)
                for (dst, par) in [(SpS, spp), (Sp3S, p3p), (SpXS, sxp), (XtS, xtp)]:
                    nc.vector.tensor_reduce(
                        out=dst[:, c:c + 1], in_=par[:], axis=X_, op=Op.add)

            # ---- assemble per-row losses
            sqS = mpool.tile([P, NCH], f32, tag="sqS")
            nc.scalar.activation(sqS[:], SpS[:], Act.Sqrt)
            den2 = mpool.tile([P, NCH], f32, tag="den2")
            nc.vector.tensor_tensor(out=den2[:], in0=SpS[:], in1=sqS[:], op=Op.mult)
            rec2 = mpool.tile([P, NCH], f32, tag="rec2")
            nc.vector.reciprocal(rec2[:], den2[:])
            q = mpool.tile([P, NCH], f32, tag="q")
            nc.vector.tensor_tensor(out=q[:], in0=Sp3S[:], in1=rec2[:], op=Op.mult)
            omega = mpool.tile([P, NCH], f32, tag="om")
            nc.vector.tensor_scalar(
                omega[:], q[:], 1.0, float(-4.0 / 3.0), op0=Op.subtract, op1=Op.mult)
            recS = mpool.tile([P, NCH], f32, tag="recS")
            nc.vector.reciprocal(recS[:], SpS[:])
            t2 = mpool.tile([P, NCH], f32, tag="t2")
            nc.vector.tensor_tensor(out=t2[:], in0=SpXS[:], in1=recS[:], op=Op.mult)
            dot = mpool.tile([P, NCH], f32, tag="dot")
            nc.vector.tensor_tensor(out=dot[:], in0=t2[:], in1=XtS[:], op=Op.subtract)
            lrow = mpool.tile([P, NCH], f32, tag="lrow")
            nc.vector.tensor_tensor(out=lrow[:], in0=omega[:], in1=dot[:], op=Op.add)
            nc.sync.dma_start(out=Ld[:].rearrange("(c p) -> p c", p=P), in_=lrow[:])
            dbg = kpool.tile([P, NCH, 8], f32, tag="dbg")
            for (i, t) in enumerate([rmaxS, tauS, tauMS, XtS, SpS, Sp3S, SpXS, omega]):
                nc.vector.tensor_copy(dbg[:, :, i], t[:])
            nc.sync.dma_start(
                out=Dbg[:].rearrange("(c p) e -> p c e", p=P), in_=dbg[:])
    nc.finalize()
    return nc


def _get_nc():
    if "nc" not in _CACHE:
        _CACHE["nc"] = _build()
    return _CACHE["nc"]


def kernel(X, target):
    from concourse.bass_utils import run_bass_kernel_spmd

    X = np.ascontiguousarray(np.asarray(X, dtype=np.float32))
    tgt = np.asarray(target).astype(np.int32)
    assert X.shape == (N, V), X.shape
    nc = _get_nc()
    in_maps = []
    for c in range(NCORES):
        in_maps.append({
            "X": X[c * RPC:(c + 1) * RPC],
            "target": np.ascontiguousarray(tgt[c * RPC:(c + 1) * RPC]),
        })
    res = run_bass_kernel_spmd(nc, in_maps, list(range(NCORES)))
    total = np.float64(0.0)
    for c in range(NCORES):
        total += np.asarray(res.results[c]["loss_rows"], dtype=np.float64).sum()
    return np.float32(total)
